# revision 40
# baseline (speedup 1.0000x reference)
"""Multi-headed attention (B=4, S=2048, D=1024, H=16) on 8 trn2 NeuronCores.

Sharding: core c handles batch b=c//2, head-half hh=c%2 (heads hh*8..hh*8+7).

Design (v4): ACT (softmax exp, ~285us/core) is the roofline; keep it
saturated from ~35us on.
  - all activations/weights are converted to bf16 on the HOST and DMA'd
    straight into SBUF (no on-device casts, half the HBM traffic).
  - minimal phase-1 head: K projection for head-pair 0 + V for kpos
    groups 0..2; the rest of K/V is emitted as fillers inside the
    attention loop, with loads prefetched a few chunks ahead.
  - scores per chunk run as a row-split tile_position pair (head 2j on
    PE rows 0:64, head 2j+1 on rows 64:128); emitted after the AV
    matmuls of an earlier chunk, the pair executes concurrently (2x).
  - AV uses ones-augmented V (M=65) to produce softmax row sums in the
    same stream; V bias is applied on the DVE at drain.
  - out projection of tile t-1 is spread one row-chunk per head-pair
    across tile t; xq for tile t+1 is prefetched during tile t.
Host: out[b] = core(2b) + core(2b+1) + bo.
"""

import ml_dtypes
import numpy as np

import concourse.tile as tile
from concourse import bacc, mybir
from concourse.bass_utils import run_bass_kernel_spmd

B, S, D, H = 4, 2048, 1024, 16
HD = D // 2          # feature columns per core (8 heads * 64)
KC = D // 128        # 8 contraction chunks over model dim
FT = HD // 128       # 4 feature tiles (head pairs)
ST = S // 512        # 4 query tiles
RT = S // 128        # 16 row tiles / S_k chunks

f32 = mybir.dt.float32
bf16 = mybir.dt.bfloat16
np_bf16 = np.dtype(ml_dtypes.bfloat16)
EXP = mybir.ActivationFunctionType.Exp

_CACHED_NC = None
_LAST_IN_MAPS = None


def build_nc():
    nc = bacc.Bacc("TRN2", target_bir_lowering=False, debug=False)

    xq_d = nc.dram_tensor("xq", (D, S), bf16, kind="ExternalInput")
    xk_d = nc.dram_tensor("xk", (D, S), bf16, kind="ExternalInput")
    xv_d = nc.dram_tensor("xv", (D, S), bf16, kind="ExternalInput")
    wq_d = nc.dram_tensor("wq", (D, HD), bf16, kind="ExternalInput")
    wk_d = nc.dram_tensor("wk", (D, HD), bf16, kind="ExternalInput")
    wv_d = nc.dram_tensor("wv", (D, HD), bf16, kind="ExternalInput")
    wo_d = nc.dram_tensor("wo", (HD, D), bf16, kind="ExternalInput")
    bqr_d = nc.dram_tensor("bqr", (128, FT), f32, kind="ExternalInput")
    bkr_d = nc.dram_tensor("bkr", (128, FT), f32, kind="ExternalInput")
    bv_d = nc.dram_tensor("bv", (1, HD), f32, kind="ExternalInput")
    o_d = nc.dram_tensor("o", (S, D), f32, kind="ExternalOutput")

    with tile.TileContext(nc) as tc:
        with (
            tc.tile_pool(name="cpool", bufs=1) as cpool,
            tc.tile_pool(name="big", bufs=1) as big,

            tc.tile_pool(name="qt", bufs=2) as qtp,
            tc.tile_pool(name="vgp", bufs=2) as vgp,
            tc.tile_pool(name="xqp", bufs=2) as xqp,
            tc.tile_pool(name="ptp", bufs=6) as ptp,
            tc.tile_pool(name="nrm", bufs=3) as nrm,
            tc.tile_pool(name="rsp", bufs=1) as rsp,
            tc.tile_pool(name="ostage", bufs=2) as ostage,
            tc.tile_pool(name="rsd", bufs=2, space="DRAM") as rsd,
            tc.tile_pool(name="psc", bufs=2, space="PSUM") as psc,
            tc.tile_pool(name="px", bufs=2, space="PSUM") as px,
            tc.tile_pool(name="scr", bufs=2, space="PSUM") as scr,
        ):
            # ---------------- constants / biases ----------------
            onecol_f = cpool.tile([128, 1], f32, name="onecol_f")
            nc.gpsimd.memset(onecol_f[:], 1.0)

            bqr_s = cpool.tile([128, FT], f32, name="bqr_s")
            nc.sync.dma_start(bqr_s[:], bqr_d[:])
            bkr_s = cpool.tile([128, FT], f32, name="bkr_s")
            nc.sync.dma_start(bkr_s[:], bkr_d[:])
            bvt_f = cpool.tile([128, HD], f32, name="bvt_f")
            nc.scalar.dma_start(bvt_f[:], bv_d[0:1, :].to_broadcast((128, HD)))
            bvt = cpool.tile([128, 8, 64], bf16, name="bvt")
            nc.gpsimd.tensor_copy(
                bvt[:], bvt_f[:].rearrange("p (h e) -> p h e", h=8))

            # ---------------- persistent big tiles ----------------
            K = big.tile([128, FT, S], bf16, name="Kfm")
            Vs = big.tile([128, RT, 8, 65], bf16, name="Vs")
            X = big.tile([128, FT, S], bf16, name="Xfm")
            wo_s = big.tile([128, FT, D], bf16, name="wo_s")
            wk_t = [big.tile([128, HD], bf16, name=f"wk{c}")
                    for c in range(KC)]
            wv_t = [big.tile([128, HD], bf16, name=f"wv{c}")
                    for c in range(KC)]
            wq_t = [big.tile([128, HD], bf16, name=f"wq{c}")
                    for c in range(KC)]
            xk_t = [big.tile([128, ST, 512], bf16, name=f"xk{c}")
                    for c in range(KC)]
            nc.vector.tensor_copy(
                Vs[:, :, :, 64:65],
                onecol_f[:, 0:1].to_broadcast((128, RT, 8, 1)),
            )

            # chunked DMAs (1KB lines pipeline at ~210GB/s per queue;
            # monolithic strided DMAs measured 2.5x slower)
            def stage_w(w_d, dst_list, qeng):
                src = w_d[:].rearrange("(k p) n -> p k n", p=128)
                for kc, dst in enumerate(dst_list):
                    qeng.dma_start(dst[:], src[:, kc, :])

            def stage_xk_kc(kc, qeng):
                # full xk rows for chunk kc: 4KB contiguous lines, all t
                qeng.dma_start(
                    xk_t[kc][:].rearrange("p t n -> p (t n)"),
                    xk_d[kc * 128:(kc + 1) * 128, :])

            def stage_xg(x_d, g, qeng, pool, tag):
                xg = pool.tile([128, KC, 512], bf16, tag=tag, name=tag)
                for kc in range(KC):
                    qeng.dma_start(
                        xg[:, kc, :],
                        x_d[kc * 128:(kc + 1) * 128,
                            g * 512:(g + 1) * 512])
                return xg

            stage_w(wk_d, wk_t, nc.sync)
            stage_w(wv_d, wv_t, nc.gpsimd)

            # ---------------- projection emitters ----------------
            def emit_kproj_mms(fts, t):
                pss = [scr.tile([128, 512], f32, tag="scr", name="pk")
                       for _ in fts]
                for kc in range(KC):
                    for ps, ft in zip(pss, fts):
                        nc.tensor.matmul(
                            ps[:],
                            wk_t[kc][:, ft * 128: (ft + 1) * 128],
                            xk_t[kc][:, t, :],
                            start=(kc == 0),
                            stop=(kc == KC - 1),
                        )
                for ps, ft in zip(pss, fts):
                    nc.vector.tensor_scalar_add(
                        K[:, ft, t * 512: (t + 1) * 512],
                        ps[:],
                        bkr_s[:, ft: ft + 1],
                    )

            def emit_vproj_rt(rt, vg):
                rr = rt % 4
                ps = scr.tile([128, 512], f32, tag="scr", name="pv")
                for kc in range(KC):
                    nc.tensor.matmul(
                        ps[:],
                        vg[:, kc, rr * 128: (rr + 1) * 128],
                        wv_t[kc][:],
                        start=(kc == 0),
                        stop=(kc == KC - 1),
                    )
                nc.vector.tensor_add(
                    Vs[:, rt, :, 0:64],
                    ps[:].rearrange("p (h e) -> p h e", h=8),
                    bvt[:],
                )

            # ---------------- phase-1 head ----------------
            # interleave V groups and K columns; all staging is a handful
            # of whole-tensor DMAs racing ahead of the PE stream
            for kc in range(KC):
                stage_xk_kc(kc, nc.sync)
            vgs = {0: stage_xg(xv_d, 0, nc.scalar, vgp, "vg")}
            vgs[1] = stage_xg(xv_d, 1, nc.gpsimd, vgp, "vg")
            stage_w(wq_d, wq_t, nc.scalar)
            xq_staged = {0: stage_xg(xq_d, 0, nc.scalar, xqp, "xq")}
            stage_w(wo_d, [wo_s[:, fc, :] for fc in range(FT)], nc.gpsimd)
            emit_kproj_mms([0, 1], 0)
            for g in range(3):
                if g >= 1:
                    vgs[g + 1] = stage_xg(
                        xv_d, g + 1, nc.gpsimd, vgp, "vg")
                for rr in range(4):
                    emit_vproj_rt(g * 4 + rr, vgs[g])
                emit_kproj_mms([0, 1], g + 1)

            # ---------------- filler schedule ----------------
            fillers = {}

            def add_fill(key, fn):
                fillers.setdefault(key, []).append(fn)

            def mm_u(fts, t):
                def f(fts=fts, t=t):
                    emit_kproj_mms(fts, t)
                return f

            # V group 3 MMs early in (0,0); rt 4g+rr needed by AV chunk 2rt
            for rr in range(4):
                add_fill((0, 0, rr),
                         lambda rr=rr: emit_vproj_rt(12 + rr, vgs[3]))
            # prefetch xq for the next tile during j==2
            for tt in range(1, ST):
                add_fill((tt - 1, 2, 5), lambda tt=tt: xq_staged.update(
                    {tt: stage_xg(xq_d, tt, nc.gpsimd, xqp, "xq")}))
            # K ft2+ft3 pair units (both scratch banks)
            add_fill((0, 0, 4), mm_u([2, 3], 0))
            add_fill((0, 1, 0), mm_u([2, 3], 1))
            add_fill((0, 1, 4), mm_u([2, 3], 2))
            add_fill((0, 2, 0), mm_u([2, 3], 3))

            # ---------------- phase 2: attention ----------------
            def emit_normalize(j2, rsj, tsl2):
                rrh = nrm.tile([128, 512], f32, tag="rr", name="rr")
                nc.vector.reciprocal_approx_fast(rrh[:], rsj[:])
                rd = rsd.tile([2, 512], f32, tag="rd", name="rd")
                for hh in range(2):
                    nc.sync.dma_start(
                        rd[hh: hh + 1, :],
                        rrh[32 * hh: 32 * hh + 1, :])
                for hh in range(2):
                    pb = 64 * hh
                    bcs = nrm.tile([128, 512], f32, tag="bcs", name="bcs")
                    nc.sync.dma_start(
                        bcs[pb: pb + 64, :],
                        rd[hh: hh + 1, :].to_broadcast((64, 512)))
                    nc.vector.tensor_mul(
                        X[pb: pb + 64, j2, tsl2],
                        X[pb: pb + 64, j2, tsl2],
                        bcs[pb: pb + 64, :],
                    )

            def emit_outproj_rt(t2, r2):
                rt = t2 * 4 + r2
                rsl = slice(rt * 128, (rt + 1) * 128)
                for n in range(2):
                    ps = scr.tile([128, 512], f32, tag="scr", name="pso")
                    nsl = slice(n * 512, (n + 1) * 512)
                    for fc in range(FT):
                        nc.tensor.matmul(
                            ps[:],
                            X[:, fc, rsl],
                            wo_s[:, fc, nsl],
                            start=(fc == 0),
                            stop=(fc == FT - 1),
                        )
                    ot = ostage.tile([128, 512], f32, tag="os", name="os")
                    nc.vector.tensor_copy(ot[:], ps[:])
                    nc.sync.dma_start(o_d[rsl, nsl], ot[:])

            def emit_av(xpA, xpB, j, pcc, ppA, ppB, stop_last):
                for hf in range(2):
                    kc = 2 * pcc + hf
                    nc.tensor.matmul(
                        xpA[:], Vs[:, kc, 2 * j, :], ppA[:, hf, :],
                        start=(kc == 0), stop=(stop_last and kc == RT - 1),
                    )
                    nc.tensor.matmul(
                        xpB[:], Vs[:, kc, 2 * j + 1, :], ppB[:, hf, :],
                        start=(kc == 0), stop=(stop_last and kc == RT - 1),
                    )

            def emit_qproj(t2, j2):
                qp = scr.tile([128, 512], f32, tag="scr", name="qp")
                for kc in range(KC):
                    nc.tensor.matmul(
                        qp[:],
                        wq_t[kc][:, j2 * 128: (j2 + 1) * 128],
                        xq_staged[t2][:, kc, :],
                        start=(kc == 0),
                        stop=(kc == KC - 1),
                    )
                Qt = qtp.tile([128, 512], bf16, tag="qt", name="qt")
                nc.vector.tensor_scalar_add(
                    Qt[:], qp[:], bqr_s[:, j2: j2 + 1])
                return Qt

            def emit_attention(t, j, tsl, Qt, pending):
                # AV pipelined two chunks behind the scores; AV is emitted
                # FIRST each chunk so the row-split scores pair lands
                # wait-free and streams concurrently
                xpA = px.tile([65, 512], f32, tag="px", name="xpA")
                xpB = px.tile([65, 512], f32, tag="px", name="xpB")
                pend = []
                for cc in range(8):
                    if len(pend) > 2:
                        emit_av(xpA, xpB, j, *pend.pop(0), False)
                    sA = psc.tile([128, 2, 512], f32, tag="sc", name="sA")
                    sB = psc.tile([128, 2, 512], f32, tag="sc", name="sB")
                    for hf in range(2):
                        kc = 2 * cc + hf
                        ksl = slice(kc * 128, (kc + 1) * 128)
                        nc.tensor.matmul(
                            sA[:, hf, :], K[0:64, j, ksl], Qt[0:64, :],
                            start=True, stop=True, tile_position=(0, 0),
                        )
                        nc.tensor.matmul(
                            sB[:, hf, :], K[64:128, j, ksl],
                            Qt[64:128, :],
                            start=True, stop=True, tile_position=(64, 0),
                        )
                    pA = ptp.tile([128, 2, 512], bf16, tag="pt", name="pA")
                    nc.scalar.activation(pA[:], sA[:], EXP, scale=0.125)
                    pB = ptp.tile([128, 2, 512], bf16, tag="pt", name="pB")
                    nc.scalar.activation(pB[:], sB[:], EXP, scale=0.125)
                    pend.append((cc, pA, pB))

                    for fill in fillers.pop((t, j, cc), ()):
                        fill()
                    # previous tile's out projection: one row-chunk per
                    # pair, placed mid-stream to spread the PE load
                    if cc == 3 and pending is not None:
                        emit_outproj_rt(pending, j)
                    # Q projection for the NEXT pair, pipelined here so the
                    # next scores group starts without a Qt dependency stall
                    if cc == 5 and (t, j) != (ST - 1, FT - 1):
                        nt, nj = (t, j + 1) if j < FT - 1 else (t + 1, 0)
                        qt_next[0] = emit_qproj(nt, nj)

                for item in pend:
                    emit_av(xpA, xpB, j, *item, True)

                # drain: unnormalized X and row sums to SBUF
                nc.vector.tensor_copy(X[0:64, j, tsl], xpA[0:64, :])
                nc.vector.tensor_copy(X[64:128, j, tsl], xpB[0:64, :])
                rsj = rsp.tile([128, 512], f32, tag="rs", name="rs")
                nc.vector.tensor_copy(rsj[0:1, :], xpA[64:65, :])
                nc.vector.tensor_copy(rsj[32:33, :], xpB[64:65, :])
                return rsj

            pending = None
            norm_pending = None
            qt_next = [emit_qproj(0, 0)]
            for t in range(ST):
                tsl = slice(t * 512, (t + 1) * 512)

                for j in range(FT):
                    Qt = qt_next[0]
                    if norm_pending is not None:
                        emit_normalize(*norm_pending)
                        norm_pending = None

                    rsj = emit_attention(t, j, tsl, Qt, pending)
                    norm_pending = (j, rsj, tsl)

                if norm_pending is not None:
                    emit_normalize(*norm_pending)
                    norm_pending = None
                pending = t
            for r2 in range(4):
                emit_outproj_rt(pending, r2)

    nc.compile()
    return nc


def kernel(**inputs):
    global _CACHED_NC, _LAST_IN_MAPS
    if _CACHED_NC is None:
        _CACHED_NC = build_nc()
    nc = _CACHED_NC

    query = np.asarray(inputs["query"], dtype=np.float32)
    key = np.asarray(inputs["key"], dtype=np.float32)
    value = np.asarray(inputs["value"], dtype=np.float32)
    fc_w = np.asarray(inputs["fc_w"], dtype=np.float32)
    Wq = np.asarray(inputs["Wq"], dtype=np.float32)
    Wk = np.asarray(inputs["Wk"], dtype=np.float32)
    Wv = np.asarray(inputs["Wv"], dtype=np.float32)
    Wo = np.asarray(inputs["Wo"], dtype=np.float32)
    bq = np.asarray(inputs["bq"], dtype=np.float32)
    bk = np.asarray(inputs["bk"], dtype=np.float32)
    bv = np.asarray(inputs["bv"], dtype=np.float32)
    bo = np.asarray(inputs["bo"], dtype=np.float32)

    wq_eff = fc_w * Wq

    def b16(a):
        return np.ascontiguousarray(a).astype(np_bf16)

    in_maps = []
    for c in range(8):
        b, hh = c // 2, c % 2
        hs = slice(hh * HD, (hh + 1) * HD)
        in_maps.append({
            "xq": b16(query[b].T),
            "xk": b16(key[b].T),
            "xv": b16(value[b].T),
            "wq": b16(wq_eff[:, hs]),
            "wk": b16(Wk[:, hs]),
            "wv": b16(Wv[:, hs]),
            "wo": b16(Wo[hs, :]),
            "bqr": np.ascontiguousarray(bq[hs].reshape(FT, 128).T),
            "bkr": np.ascontiguousarray(bk[hs].reshape(FT, 128).T),
            "bv": bv[None, hs],
        })

    _LAST_IN_MAPS = in_maps
    res = run_bass_kernel_spmd(nc, in_maps, core_ids=list(range(8)))

    out = np.empty((B, S, D), dtype=np.float32)
    for b in range(B):
        out[b] = res.results[2 * b]["o"] + res.results[2 * b + 1]["o"] + bo
    return out
